# revision 1
# baseline (speedup 1.0000x reference)
"""GCN layer (message passing) on 8 trn2 NeuronCores.

  out = relu(segment_sum(norm * (H@W.T + b)[col], row)),  norm = d^-1/2[row] d^-1/2[col]
  with self-loops appended; d = 1 + in-degree.

Strategy (SPMD over 8 cores, nodes sharded by destination):
  - Host: pad N to 100352 = 8*12544; partition edges by dest core; per dest
    block (128 nodes) group edges by source bank (4 banks of 25088 rows so
    int16 dma_gather indices fit); fixed budget PB chunks of 128 edges per
    (block, bank) -> uniform SPMD program.
  - Device phase 1: Hl2 = (H @ W.T + b) * d^-1/2 for own shard (PE matmul,
    transposed H from host). Keep f32 copy in SBUF (self-loop term), cast
    bf16 -> DRAM shard.
  - Phase 2: AllGather bf16 shards -> full 100352x128 node table per core.
  - Phase 3: per super-block of 7 dest blocks: batched dma_gather per source
    bank; selection matrix S[e,m] = (dk[e]==m) via DVE is_equal; PE matmul
    S^T @ G accumulates scatter-add into PSUM; epilogue adds the self-loop
    term and applies relu((acc + Hl2_own) * d^-1/2[dst]).
  - Self-loops never enter the edge stream (handled exactly in the epilogue).
"""
import numpy as np

N = 100000
D = 128
NCORES = 8
P = 128
NPAD = 100352            # 8 * 12544, also 4 * 25088
NPC = NPAD // NCORES     # 12544 nodes per core
NBLK = NPC // P          # 98 dest blocks per core
NBANKS = 4
BANK = NPAD // NBANKS    # 25088 rows per bank (< 2^15 for int16 idx)
def _sbb(PB):
    # dest blocks per super-block, capped so one dma_gather stays <=1024 idx
    return max(1, 8 // PB)

KDTYPE = "bf16"          # gather-table dtype: "bf16" | "f32"


# ----------------------------------------------------------------- host prep

def _host_prep(H, edge_index, W, b, PB):
    """Build per-core device inputs. PB = chunks per (block, bank)."""
    f32 = np.float32
    CPB = NBANKS * PB
    SBB = _sbb(PB)
    assert SBB * PB * P <= 1024
    NSB = (NBLK + SBB - 1) // SBB
    IPG = SBB * PB                     # chunks per gather instruction
    row = np.asarray(edge_index[0], dtype=np.int64)
    col = np.asarray(edge_index[1], dtype=np.int64)
    H = np.asarray(H, dtype=f32)
    W = np.asarray(W, dtype=f32)
    b = np.asarray(b, dtype=f32)

    deg = (1.0 + np.bincount(row, minlength=NPAD)).astype(f32)  # pad nodes: 1

    Hpad = np.zeros((NPAD, D), dtype=f32)
    Hpad[:N] = H

    HALF = NPC // 2
    HBLK = NBLK // 2
    core = row // NPC
    block = (row % NPC) // P
    dk_all = (row % NPC) % P
    c_src = col // NPC
    r_src = col % NPC
    # bank = (source half, source core-group): gathers from half h only
    # depend on the h-th half-AllGather. Shard halves are stored
    # partition-major: gather row within a core's half = p * HBLK + lt.
    rr = r_src % HALF
    bank = 2 * (r_src // HALF) + (c_src // 4)
    lidx = (c_src % 4) * HALF + (rr % P) * HBLK + (rr // P)

    gsz = np.zeros((NCORES, NBLK, NBANKS), dtype=np.int64)
    np.add.at(gsz, (core, block, bank), 1)
    if gsz.max() > PB * P:
        return None  # caller bumps PB

    order = np.lexsort((col, bank, block, core))
    sc, sb_, sk = core[order], block[order], bank[order]
    gid = (sc * NBLK + sb_) * NBANKS + sk
    starts = np.zeros(NCORES * NBLK * NBANKS, dtype=np.int64)
    np.cumsum(gsz.reshape(-1)[:-1], out=starts[1:])
    rank = np.arange(len(order)) - starts[gid]

    slots_idx = np.zeros((NCORES, NBLK, NBANKS, PB * P), dtype=np.int64)
    slots_dk = np.full((NCORES, NBLK, NBANKS, PB * P), -1.0, dtype=f32)
    slots_idx[sc, sb_, sk, rank] = lidx[order]
    slots_dk[sc, sb_, sk, rank] = dk_all[order]

    # dkT: [core, p, t*CPB + k*PB + j]
    dk4 = slots_dk.reshape(NCORES, NBLK, NBANKS, PB, P)
    dkT = np.ascontiguousarray(
        dk4.transpose(0, 4, 1, 2, 3).reshape(NCORES, P, NBLK * CPB))

    # idx16: per instruction (sb, k), position i=(lt*PB+j)*128+p, wrapped by 16
    parts = []
    for sb in range(NSB):
        nb = min(SBB, NBLK - sb * SBB)
        for k in range(NBANKS):
            arr = slots_idx[:, sb * SBB:sb * SBB + nb, k, :]  # [c, nb, PB*128]
            arr = arr.reshape(NCORES, nb * PB * P)
            parts.append(arr.reshape(NCORES, -1, 16).transpose(0, 2, 1))
    w16 = np.concatenate(parts, axis=2)                       # [c, 16, cols]
    idx16 = np.tile(w16, (1, 8, 1)).astype(np.int16)

    degT = np.ascontiguousarray(
        deg.reshape(NCORES, NBLK, P).transpose(0, 2, 1))

    WT = np.ascontiguousarray(W.T)              # [in, out]
    biasB = np.tile(b[None, :], (P, 1)).astype(f32)
    import ml_dtypes
    idt = ml_dtypes.bfloat16 if KDTYPE == "bf16" else f32
    iota = np.tile(np.arange(P, dtype=idt)[None, :], (P, 1))

    in_maps = []
    for c in range(NCORES):
        HT = np.ascontiguousarray(Hpad[c * NPC:(c + 1) * NPC].T)  # [D, NPC]
        in_maps.append(dict(
            HT=HT,
            WT=WT,
            biasB=biasB,
            iota=iota,
            degT=np.ascontiguousarray(degT[c]),
            dkT=np.ascontiguousarray(dkT[c]),
            idx16=np.ascontiguousarray(idx16[c]),
        ))
    return in_maps


# ------------------------------------------------------------- numpy device sim

def _sim_spmd(in_maps, PB):
    """Numpy mirror of the device program (for index-plumbing validation)."""
    import ml_dtypes
    f32 = np.float32
    CPB = NBANKS * PB
    SBB = _sbb(PB)
    NSB = (NBLK + SBB - 1) // SBB
    IPG = SBB * PB
    bf16 = ml_dtypes.bfloat16

    # phase 1 per core
    shards = []
    hl2own_all = []
    dis_all = []
    for c in range(NCORES):
        m = in_maps[c]
        dis = 1.0 / np.sqrt(m["degT"])                       # [p, t]
        hl2own = np.zeros((P, NBLK, D), dtype=f32)
        for t in range(NBLK):
            hl = m["HT"][:, t * P:(t + 1) * P].T @ m["WT"] + m["biasB"]
            hl2own[:, t, :] = hl * dis[:, t:t + 1]
        hl2own_all.append(hl2own)
        dis_all.append(dis)

    HALF = NPC // 2
    HBLK = NBLK // 2
    table_h = []
    for h in range(2):
        rows = []
        for c in range(NCORES):
            sl = hl2own_all[c][:, h * HBLK:(h + 1) * HBLK, :]  # [P, HBLK, D]
            rows.append(sl.reshape(HALF, D))                   # row = p*HBLK+lt
        th = np.concatenate(rows, axis=0)
        table_h.append(th.astype(bf16) if KDTYPE == "bf16" else th)

    outs = []
    for c in range(NCORES):
        m = in_maps[c]
        dis = dis_all[c]
        out_c = np.zeros((NPC, D), dtype=f32)
        idx16 = m["idx16"]
        dkT = m["dkT"]
        iota = m["iota"]
        cursor = 0
        for sb in range(NSB):
            nb = min(SBB, NBLK - sb * SBB)
            G = {}
            for k in range(NBANKS):
                ncols = nb * PB * 8
                cols = idx16[:16, cursor:cursor + ncols]     # [16, ncols]
                cursor += ncols
                idx = cols.T.reshape(-1).astype(np.int64)    # i-ordered
                h, g = k // 2, k % 2
                bank_tbl = table_h[h][g * BANK:(g + 1) * BANK]
                G[k] = bank_tbl[idx].reshape(nb * PB, P, D).transpose(1, 0, 2)
            for lt in range(nb):
                t = sb * SBB + lt
                acc = np.zeros((P, D), dtype=f32)
                for cch in range(CPB):
                    k, j = cch // PB, cch % PB
                    w = lt * PB + j
                    dk = dkT[:, t * CPB + cch]
                    S = (iota == dk[:, None]).astype(
                        bf16 if KDTYPE == "bf16" else f32)
                    acc += S.astype(f32).T @ G[k][:, w, :].astype(f32)
                self_term = hl2own_all[c][:, t, :]
                if KDTYPE == "bf16":
                    self_term = self_term.astype(bf16).astype(f32)
                tmp = acc + self_term
                out_c[t * P:(t + 1) * P] = np.maximum(tmp * dis[:, t:t + 1], 0.0)
        outs.append(out_c)
    return np.concatenate(outs, axis=0)[:N]


# ------------------------------------------------------------- device kernel

_NC_CACHE = {}
_PHASE = 3          # debug: 1 = linear only, 2 = +AllGather, 3 = full
_SHARED_AG = False  # AllGather output in Shared addr space
_P3_MODE = "full"   # debug: "full" | "nogather" | "gatheronly"
_TIMING_SINGLE = False  # single-core build for TimelineSim (no collective)


def _build_nc(PB):
    import concourse.bacc as bacc
    import concourse.mybir as mybir
    import concourse.tile as tile
    from concourse import library_config

    CPB = NBANKS * PB
    SBB = _sbb(PB)
    IPG = SBB * PB
    kdt = mybir.dt.bfloat16 if KDTYPE == "bf16" else mybir.dt.float32
    f32 = mybir.dt.float32

    nc = bacc.Bacc("TRN2", target_bir_lowering=False, debug=False,
                   num_devices=1 if _TIMING_SINGLE else NCORES)

    HT = nc.dram_tensor("HT", [D, NPC], f32, kind="ExternalInput").ap()
    WT = nc.dram_tensor("WT", [D, D], f32, kind="ExternalInput").ap()
    biasB = nc.dram_tensor("biasB", [P, D], f32, kind="ExternalInput").ap()
    iota = nc.dram_tensor("iota", [P, P], kdt, kind="ExternalInput").ap()
    degT = nc.dram_tensor("degT", [P, NBLK], f32, kind="ExternalInput").ap()
    dkT = nc.dram_tensor("dkT", [P, NBLK * CPB], f32, kind="ExternalInput").ap()
    idx16 = nc.dram_tensor("idx16", [P, NBLK * NBANKS * PB * 8], mybir.dt.int16,
                           kind="ExternalInput").ap()
    out = nc.dram_tensor("out", [NPC, D], f32, kind="ExternalOutput").ap()

    with tile.TileContext(nc) as tc:
        with (
            tc.tile_pool(name="const", bufs=1) as const,
            tc.tile_pool(name="big", bufs=1) as big,
            tc.tile_pool(name="dram", bufs=1, space="DRAM") as dram,
        ):
            nc.gpsimd.load_library(library_config.mlp)

            WT_s = const.tile([D, D], f32)
            nc.sync.dma_start(out=WT_s[:], in_=WT[:])
            biasB_s = const.tile([P, D], f32)
            nc.sync.dma_start(out=biasB_s[:], in_=biasB[:])
            iota_s = const.tile([P, P], kdt)
            nc.sync.dma_start(out=iota_s[:], in_=iota[:])
            degT_s = const.tile([P, NBLK], f32)
            nc.sync.dma_start(out=degT_s[:], in_=degT[:])

            # dis = 1/sqrt(deg)
            rec_s = const.tile([P, NBLK], f32)
            nc.vector.reciprocal(out=rec_s[:], in_=degT_s[:])
            disT_s = const.tile([P, NBLK], f32)
            nc.scalar.sqrt(out=disT_s[:], in_=rec_s[:])

            HT_s = big.tile([D, NPC], f32)
            if _PHASE >= 3:
                dkT_s = big.tile([P, NBLK * CPB], f32)
                nc.scalar.dma_start(out=dkT_s[:], in_=dkT[:])
                idx_s = big.tile([P, NBLK * NBANKS * PB * 8], mybir.dt.int16)
                nc.scalar.dma_start(out=idx_s[:], in_=idx16[:])
            # self-loop term kept as bf16 group tiles (one per GRP blocks)

            HALF = NPC // 2
            HBLK = NBLK // 2
            shard_h = [dram.tile([HALF, D], kdt, name=f"shard_h{h}")
                       for h in range(2)]
            table_h = [dram.tile([NCORES * HALF, D], kdt, name=f"table_h{h}")
                       for h in range(2)]

            # ---------------- phase 1 + per-half AllGather
            # batch shard stores per GRP blocks; issue each half's collective
            # as soon as its 49 blocks are stored -> h0 gathers overlap the
            # second half of phase 1 and the second collective
            GRP = 7
            with (
                tc.tile_pool(name="p1psum", bufs=4, space="PSUM") as p1psum,
                tc.tile_pool(name="p1sbuf", bufs=3) as p1sbuf,
            ):
                hl2own_s = {}
                for h in range(2):
                    for g0 in range(h * HBLK, (h + 1) * HBLK, GRP):
                        gn = min(GRP, (h + 1) * HBLK - g0)
                        eng = nc.sync if (g0 // GRP) % 2 == 0 else nc.scalar
                        eng.dma_start(
                            out=HT_s[:, g0 * P:(g0 + gn) * P],
                            in_=HT[:, g0 * P:(g0 + gn) * P])
                        stg = big.tile([P, GRP * D], kdt, name=f"stg_{g0}")
                        hl2own_s[g0 // GRP] = stg
                        for lt in range(gn):
                            t = g0 + lt
                            ps = p1psum.tile([P, D], f32, space="PSUM",
                                             tag="ps", name=f"ps_{t}")
                            # bias preloaded into PSUM; matmul accumulates
                            nc.vector.tensor_copy(out=ps[:], in_=biasB_s[:])
                            nc.tensor.matmul(
                                out=ps[:], lhsT=HT_s[:, t * P:(t + 1) * P],
                                rhs=WT_s[:], start=False, stop=True)
                            # mult by dis with bf16 output = fused cast
                            nc.vector.tensor_scalar(
                                out=stg[:, lt * D:(lt + 1) * D],
                                in0=ps[:],
                                scalar1=disT_s[:, t:t + 1], scalar2=None,
                                op0=mybir.AluOpType.mult)
                        lb = g0 - h * HBLK
                        # partition-major shard layout: flat row = p*HBLK+lt,
                        # so this store is per-partition contiguous
                        eng.dma_start(
                            out=shard_h[h][:].rearrange(
                                "(p l) f -> p (l f)", p=P)[:, lb * D:(lb + gn) * D],
                            in_=stg[:, :gn * D])
                    if _PHASE >= 2:
                        if _TIMING_SINGLE:
                            # cost-model stand-in: same local HBM write volume
                            for c in range(NCORES):
                                nc.gpsimd.dma_start(
                                    out=table_h[h][c * HALF:(c + 1) * HALF, :],
                                    in_=shard_h[h][:])
                        else:
                            nc.gpsimd.collective_compute(
                                "AllGather", mybir.AluOpType.bypass,
                                replica_groups=[list(range(NCORES))],
                                ins=[shard_h[h].opt()],
                                outs=[table_h[h].opt()],
                            )

            if _PHASE == 2:
                # debug: sample 1568 rows of table_h[0] from every core region
                with tc.tile_pool(name="p2chk", bufs=3) as p2chk:
                    for c in range(NCORES):
                        for q in range(1568 // P):
                            src = c * HALF + q * P
                            ck = p2chk.tile([P, D], kdt, tag="ck")
                            nc.sync.dma_start(
                                out=ck[:],
                                in_=table_h[0][src:src + P, :])
                            ckf = p2chk.tile([P, D], f32, tag="ckf")
                            nc.vector.tensor_copy(out=ckf[:], in_=ck[:])
                            dst = (c * 1568 + q * P)
                            nc.sync.dma_start(
                                out=out[dst:dst + P, :], in_=ckf[:])

            # ---------------- phase 3: edge aggregation
            if _PHASE >= 3:
                _run_phase3(nc, tc, mybir, tile, PB, kdt, f32,
                            table_h, dkT_s, idx_s, iota_s, disT_s,
                            hl2own_s, out)

    nc.finalize()
    return nc


def _run_phase3(nc, tc, mybir, tile, PB, kdt, f32, table_h, dkT_s, idx_s,
                iota_s, disT_s, hl2own_s, out):
    CPB = NBANKS * PB
    SBB = _sbb(PB)
    NSB = (NBLK + SBB - 1) // SBB
    if True:
        if True:
            with (
                tc.tile_pool(name="gpool", bufs=3 * NBANKS) as gpool,
                tc.tile_pool(name="spool", bufs=4) as spool,
                tc.tile_pool(name="acc", bufs=min(8, 2 * SBB), space="PSUM") as accp,
                tc.tile_pool(name="epi", bufs=4) as epi,
            ):
                cursor = 0
                for sb in range(NSB):
                    nb = min(SBB, NBLK - sb * SBB)
                    G = []
                    for k in range(NBANKS):
                        nidx = nb * PB * P
                        g = gpool.tile([P, nb * PB, D], kdt, tag="g",
                                       name=f"g_{sb}_{k}")
                        if _P3_MODE == "nogather":
                            nc.vector.memset(g[:], 1.0)
                        else:
                            h, gg = k // 2, k % 2
                            nc.gpsimd.dma_gather(
                                g[:],
                                table_h[h][gg * BANK:(gg + 1) * BANK, :],
                                idx_s[:, cursor:cursor + nidx // 16],
                                nidx, nidx, D)
                        cursor += nidx // 16
                        G.append(g)
                    if _P3_MODE == "gatheronly":
                        continue  # crash-isolation mode; output unchecked
                    accs = [accp.tile([P, D], f32, space="PSUM", tag="acc",
                                      name=f"acc_{sb}_{i}")
                            for i in range(nb)]
                    for lt in range(nb):
                        t = sb * SBB + lt
                        for cch in range(CPB):
                            k, j = cch // PB, cch % PB
                            w = lt * PB + j
                            S = spool.tile([P, P], kdt, tag="s")
                            nc.vector.tensor_scalar(
                                out=S[:], in0=iota_s[:],
                                scalar1=dkT_s[:, t * CPB + cch:t * CPB + cch + 1],
                                scalar2=None, op0=mybir.AluOpType.is_equal)
                            nc.tensor.matmul(
                                out=accs[lt][:], lhsT=S[:],
                                rhs=G[k][:, w, :],
                                start=(cch == 0), stop=(cch == CPB - 1))
                    ostg = epi.tile([P, SBB * D], f32, tag="ostg")
                    for lt in range(nb):
                        t = sb * SBB + lt
                        tmp = epi.tile([P, D], f32, tag="tmp")
                        nc.vector.tensor_tensor(
                            out=tmp[:], in0=accs[lt][:],
                            in1=hl2own_s[t // 7][:, (t % 7) * D:
                                                 (t % 7 + 1) * D],
                            op=mybir.AluOpType.add)
                        nc.scalar.activation(
                            out=ostg[:, lt * D:(lt + 1) * D], in_=tmp[:],
                            func=mybir.ActivationFunctionType.Relu,
                            scale=disT_s[:, t:t + 1])
                    eng = nc.sync if sb % 2 == 0 else nc.scalar
                    # out is partition-major (flat row = p*NBLK + t); host
                    # reorders to node-major after download
                    eng.dma_start(
                        out=out[:].rearrange(
                            "(p t) f -> p (t f)", p=P)[:, sb * SBB * D:
                                                       (sb * SBB + nb) * D],
                        in_=ostg[:, :nb * D])


def kernel(H, edge_index, W, b):
    from concourse.bass_utils import run_bass_kernel_spmd

    PB = 2
    in_maps = None
    while in_maps is None:
        in_maps = _host_prep(H, edge_index, W, b, PB)
        if in_maps is None:
            PB += 1

    if PB not in _NC_CACHE:
        _NC_CACHE[PB] = _build_nc(PB)
    nc = _NC_CACHE[PB]

    res = run_bass_kernel_spmd(nc, in_maps, list(range(NCORES)))
    # device out is partition-major: flat row = p*NBLK + t -> node t*128+p
    outs = []
    for c in range(NCORES):
        o = res.results[c]["out"].reshape(P, NBLK, D)
        outs.append(o.transpose(1, 0, 2).reshape(NPC, D))
    out = np.concatenate(outs, axis=0)
    return np.ascontiguousarray(out[:N])



# revision 4
# speedup vs baseline: 6.9579x; 6.9579x over previous
"""GCN layer (message passing) on 8 trn2 NeuronCores.

  out = relu(segment_sum(norm * (H@W.T + b)[col], row)),  norm = d^-1/2[row] d^-1/2[col]
  with self-loops appended; d = 1 + in-degree.

Strategy (SPMD over 8 cores, edges partitioned by destination on host):
  - Aggregate-then-transform (GCN linearity):
      out[r] = relu(dis[r] * (Z[r] @ W.T + sigma[r] * b)),
      Z[r] = sum_{e: dst=r} dis[col_e] * H[col_e],  sigma[r] = sum_e dis[col_e]
    (sums include the self-loop edge r->r).
  - Host: shard nodes contiguously (12500/core), bin-pack each core's nodes
    into 98 blocks of 128 balancing per-block message counts; lay out each
    block's messages as CPB chunks of 128 edge slots; ship pre-scaled source
    features Hexp[e] = dis[col]*H[col] (bf16) in chunk-major [slot, chunk*D]
    layout plus per-slot dest keys dkT, per-node sigT/disT, W^T, bias.
  - Device per chunk: S = (iota == dk) one-hot (DVE); zt += Hexp_chunk^T @ S
    (PE, PSUM accum over the block's CPB chunks).  Per block: ztb = bf16(zt)
    (ACT); acc = sigma*bias (DVE preload) + ztb^T... acc += ztb.T @ W^T via
    PE (start=False); out_block = relu(acc * dis) (ACT) -> DMA.
  - No collectives, no dma_gather: GPSIMD stays idle; all DMA is bulk
    contiguous HWDGE.
"""
import numpy as np

N = 100000
D = 128
NCORES = 8
P = 128
NPC_REAL = N // NCORES          # 12500 real nodes per core
NBLK = 98                       # blocks of 128 slots (12544 slots, 44 dummy)
NPC = NBLK * P                  # 12544 slots per core


# ----------------------------------------------------------------- host prep

def _pack_blocks(deg_n, cpb):
    """Bin-pack all N nodes into 784 bins (<=128 each) balancing deg sums.

    deg_n: [N] message counts (in-degree + 1).  Returns [N] bin ids with
    every bin's deg sum <= cpb*128, or None if not achievable.
    """
    cap = cpb * P
    nbins = NCORES * NBLK
    order = np.argsort(-deg_n, kind="stable")
    bins = -np.ones(len(deg_n), dtype=np.int64)
    # snake placement of descending degrees
    for r in range(0, len(order), nbins):
        chunk = order[r:r + nbins]
        ids = np.arange(len(chunk))
        if (r // nbins) % 2 == 1:
            ids = nbins - 1 - ids
        bins[chunk] = ids[:len(chunk)]
    sums = np.bincount(bins, weights=deg_n, minlength=nbins).astype(np.int64)
    cnts = np.bincount(bins, minlength=nbins)
    # greedy fixup: swap items between fullest and emptiest bins
    for _ in range(20000):
        a = int(np.argmax(sums))
        if sums[a] <= cap:
            break
        b = int(np.argmin(sums))
        need = int(sums[a] - cap)
        room = int(cap - sums[b])
        ia = np.where(bins == a)[0]
        ib = np.where(bins == b)[0]
        da, db = deg_n[ia], deg_n[ib]
        # swap (i from a, j from b): diff = da_i - db_j must be >=1 and
        # <= room; prefer the smallest diff >= need, else the largest <= room
        diff = da[:, None] - db[None, :]
        ok = (diff >= 1) & (diff <= room)
        if not ok.any():
            return None
        dd = np.where(ok, diff, -1)
        best = np.where((dd >= min(need, room)) & ok, dd,
                        np.iinfo(np.int64).max)
        if best.min() != np.iinfo(np.int64).max:
            i, j = np.unravel_index(np.argmin(best), best.shape)
        else:
            i, j = np.unravel_index(np.argmax(dd), dd.shape)
        d = int(diff[i, j])
        bins[ia[i]], bins[ib[j]] = b, a
        sums[a] -= d
        sums[b] += d
    if sums.max() > cap or cnts.max() > P:
        return None
    return bins


def _host_prep(H, edge_index, W, b, CPB):
    """Build per-core device inputs; None if CPB chunks/block don't fit."""
    import ml_dtypes
    f32 = np.float32
    bf16 = ml_dtypes.bfloat16
    NC = NBLK * CPB                  # chunks per core

    row = np.asarray(edge_index[0], dtype=np.int64)
    col = np.asarray(edge_index[1], dtype=np.int64)
    H = np.asarray(H, dtype=f32)
    W = np.asarray(W, dtype=f32)
    b = np.asarray(b, dtype=f32)

    deg = (1.0 + np.bincount(row, minlength=N)).astype(f32)
    dis = (1.0 / np.sqrt(deg)).astype(f32)
    # sigma[r] = sum over messages into r (incl self) of dis[col]
    sig = np.bincount(row, weights=dis[col].astype(np.float64),
                      minlength=N).astype(f32) + dis

    disH = (dis[:, None] * H).astype(bf16)        # pre-scaled source features

    deg_i = deg.astype(np.int64)

    iota = np.tile(np.arange(P, dtype=bf16)[None, :], (P, 1))
    WTb = np.ascontiguousarray(W.T).astype(bf16)  # [d, f]
    biasB = np.tile(b[None, :], (P, 1)).astype(f32)

    bins = _pack_blocks(deg_i, CPB)               # global: 784 bins
    if bins is None:
        return None, None
    # node -> global slot: bin b is (core b//NBLK, block b%NBLK)
    order = np.argsort(bins, kind="stable")
    bin_of = bins[order]
    slot_of = np.arange(N) - np.searchsorted(bin_of, bin_of)
    node_pos = np.empty(N, dtype=np.int64)        # core*NPC + blk*P + slot
    node_pos[order] = bin_of * P + slot_of

    allr = np.concatenate([row, np.arange(N, dtype=np.int64)])  # + self loops
    allc = np.concatenate([col, np.arange(N, dtype=np.int64)])
    dst = node_pos[allr]
    ecore = dst // NPC

    in_maps = []
    for c in range(NCORES):
        em = ecore == c
        ec = allc[em]
        dstc = dst[em] - c * NPC
        dblk = dstc // P
        dk = dstc % P
        eorder = np.argsort(dblk, kind="stable")
        dblk_s = dblk[eorder]
        rank = np.arange(len(eorder)) - np.searchsorted(dblk_s, dblk_s)
        assert rank.max() < CPB * P
        cidx = dblk_s * CPB + rank // P              # chunk index
        slot = rank % P

        Hexp3 = np.zeros((NC, P, D), dtype=bf16)
        Hexp3[cidx, slot] = disH[ec[eorder]]
        HexpT = np.ascontiguousarray(
            Hexp3.transpose(1, 0, 2).reshape(P, NC * D))

        dkT = np.full((P, NC), -1.0, dtype=f32)
        dkT[slot, cidx] = dk[eorder].astype(f32)

        nm = (node_pos >= c * NPC) & (node_pos < (c + 1) * NPC)
        npos = node_pos[nm] - c * NPC
        sigT = np.zeros((P, NBLK), dtype=f32)
        disT = np.zeros((P, NBLK), dtype=f32)
        sigT[npos % P, npos // P] = sig[nm]
        disT[npos % P, npos // P] = dis[nm]

        in_maps.append(dict(
            Hexp=HexpT, dkT=np.ascontiguousarray(dkT),
            sigT=np.ascontiguousarray(sigT),
            disT=np.ascontiguousarray(disT),
            WTb=WTb, biasB=biasB, iota=iota,
        ))
    return in_maps, node_pos


# ------------------------------------------------------------- numpy device sim

def _sim_spmd(in_maps, CPB):
    """Numpy mirror of the device program (index-plumbing validation)."""
    import ml_dtypes
    f32 = np.float32
    bf16 = ml_dtypes.bfloat16
    outs = []
    for m in in_maps:
        Hexp = m["Hexp"].astype(f32)          # [128, NC*D]
        dkT = m["dkT"]
        iota = m["iota"].astype(f32)
        out_c = np.zeros((P, NBLK, D), dtype=f32)
        for t in range(NBLK):
            zt = np.zeros((D, P), dtype=f32)  # [d, dst]
            for j in range(CPB):
                c = t * CPB + j
                S = (iota == dkT[:, c:c + 1]).astype(f32)   # [e, dst]
                He = Hexp[:, c * D:(c + 1) * D]             # [e, d]
                zt += He.T @ S
            ztb = zt.astype(bf16).astype(f32)
            acc = m["biasB"] * m["sigT"][:, t:t + 1]        # [dst, f]
            acc = acc + ztb.T @ m["WTb"].astype(f32)
            out_c[:, t, :] = np.maximum(acc * m["disT"][:, t:t + 1], 0.0)
        outs.append(out_c.transpose(1, 0, 2).reshape(NPC, D))
    return outs


# ------------------------------------------------------------- device kernel

_NC_CACHE = {}
_LAST = {}          # exposes (nc, in_maps, CPB) of last kernel() call
GRP = 7             # dest blocks per DMA group (98 = 14*7)
SDVE = 7            # S-builds per block on DVE (rest on ACT)


def _build_nc(CPB):
    import concourse.bacc as bacc
    import concourse.mybir as mybir
    import concourse.tile as tile

    bf = mybir.dt.bfloat16
    f32 = mybir.dt.float32
    NC = NBLK * CPB

    nc = bacc.Bacc("TRN2", target_bir_lowering=False, debug=False,
                   num_devices=NCORES)

    Hexp = nc.dram_tensor("Hexp", [P, NC * D], bf, kind="ExternalInput").ap()
    dkT = nc.dram_tensor("dkT", [P, NC], f32, kind="ExternalInput").ap()
    sigT = nc.dram_tensor("sigT", [P, NBLK], f32, kind="ExternalInput").ap()
    disT = nc.dram_tensor("disT", [P, NBLK], f32, kind="ExternalInput").ap()
    WTb = nc.dram_tensor("WTb", [P, D], bf, kind="ExternalInput").ap()
    biasB = nc.dram_tensor("biasB", [P, D], f32, kind="ExternalInput").ap()
    iota = nc.dram_tensor("iota", [P, P], bf, kind="ExternalInput").ap()
    out = nc.dram_tensor("out", [NPC, D], f32, kind="ExternalOutput").ap()

    with tile.TileContext(nc) as tc:
        with (
            tc.tile_pool(name="const", bufs=1) as const,
            tc.tile_pool(name="hexp", bufs=3) as hpool,
            tc.tile_pool(name="spool", bufs=6) as spool,
            tc.tile_pool(name="ztb", bufs=4) as ztbpool,
            tc.tile_pool(name="ostg", bufs=3) as opool,
            tc.tile_pool(name="zt", bufs=4, space="PSUM") as ztpool,
            tc.tile_pool(name="acc", bufs=4, space="PSUM") as accpool,
        ):
            WTb_s = const.tile([P, D], bf)
            nc.sync.dma_start(out=WTb_s[:], in_=WTb[:])
            biasB_s = const.tile([P, D], f32)
            nc.sync.dma_start(out=biasB_s[:], in_=biasB[:])
            iota_s = const.tile([P, P], bf)
            nc.sync.dma_start(out=iota_s[:], in_=iota[:])
            sigT_s = const.tile([P, NBLK], f32)
            nc.sync.dma_start(out=sigT_s[:], in_=sigT[:])
            disT_s = const.tile([P, NBLK], f32)
            nc.sync.dma_start(out=disT_s[:], in_=disT[:])
            dkT_s = const.tile([P, NC], f32)
            nc.scalar.dma_start(out=dkT_s[:], in_=dkT[:])

            for g in range(NBLK // GRP):
                eng = nc.sync if g % 2 == 0 else nc.scalar
                hx = hpool.tile([P, GRP * CPB * D], bf, tag="hx",
                                name=f"hx_{g}")
                eng.dma_start(
                    out=hx[:],
                    in_=Hexp[:, g * GRP * CPB * D:(g + 1) * GRP * CPB * D])
                ostg = opool.tile([P, GRP * D], f32, tag="o", name=f"o_{g}")
                for lt in range(GRP):
                    t = g * GRP + lt
                    zt = ztpool.tile([P, P], f32, space="PSUM", tag="zt",
                                     name=f"zt_{t}")
                    for j in range(CPB):
                        c = t * CPB + j
                        S = spool.tile([P, P], bf, tag="s")
                        nc.vector.tensor_scalar(
                            out=S[:], in0=iota_s[:],
                            scalar1=dkT_s[:, c:c + 1], scalar2=None,
                            op0=mybir.AluOpType.is_equal)
                        nc.tensor.matmul(
                            out=zt[:],
                            lhsT=hx[:, (lt * CPB + j) * D:(lt * CPB + j + 1) * D],
                            rhs=S[:], start=(j == 0), stop=(j == CPB - 1))
                    ztb = ztbpool.tile([P, P], bf, tag="ztb")
                    nc.scalar.copy(out=ztb[:], in_=zt[:])
                    acc = accpool.tile([P, D], f32, space="PSUM", tag="acc")
                    nc.vector.tensor_scalar(
                        out=acc[:], in0=biasB_s[:],
                        scalar1=sigT_s[:, t:t + 1], scalar2=None,
                        op0=mybir.AluOpType.mult)
                    nc.tensor.matmul(out=acc[:], lhsT=ztb[:], rhs=WTb_s[:],
                                     start=False, stop=True)
                    nc.scalar.activation(
                        out=ostg[:, lt * D:(lt + 1) * D], in_=acc[:],
                        func=mybir.ActivationFunctionType.Relu,
                        scale=disT_s[:, t:t + 1])
                eng2 = nc.scalar if g % 2 == 0 else nc.sync
                # out is partition-major (flat row = p*NBLK + t)
                eng2.dma_start(
                    out=out[:].rearrange(
                        "(p t) f -> p (t f)", p=P)[:, g * GRP * D:
                                                   (g + 1) * GRP * D],
                    in_=ostg[:])

    nc.finalize()
    return nc


def kernel(H, edge_index, W, b):
    from concourse.bass_utils import run_bass_kernel_spmd

    CPB = 7
    in_maps, node_pos = _host_prep(H, edge_index, W, b, CPB)
    if in_maps is None:
        CPB = 8
        in_maps, node_pos = _host_prep(H, edge_index, W, b, CPB)
        assert in_maps is not None

    if CPB not in _NC_CACHE:
        _NC_CACHE[CPB] = _build_nc(CPB)
    nc = _NC_CACHE[CPB]
    _LAST.update(nc=nc, in_maps=in_maps, CPB=CPB)

    res = run_bass_kernel_spmd(nc, in_maps, list(range(NCORES)))
    # device out is partition-major: flat row = p*NBLK + t -> slot (t, p)
    full = np.empty((NCORES * NPC, D), dtype=np.float32)
    for c in range(NCORES):
        o = res.results[c]["out"].reshape(P, NBLK, D)
        full[c * NPC:(c + 1) * NPC] = o.transpose(1, 0, 2).reshape(NPC, D)
    return np.ascontiguousarray(full[node_pos])


# revision 6
# speedup vs baseline: 7.4233x; 1.0669x over previous
"""GCN layer (message passing) on 8 trn2 NeuronCores.

  out = relu(segment_sum(norm * (H@W.T + b)[col], row)),  norm = d^-1/2[row] d^-1/2[col]
  with self-loops appended; d = 1 + in-degree.

Strategy (SPMD over 8 cores, edges partitioned by destination on host):
  - Aggregate-then-transform (GCN linearity):
      out[r] = relu(dis[r] * (Z[r] @ W.T + sigma[r] * b)),
      Z[r] = sum_{e: dst=r} dis[col_e] * H[col_e],  sigma[r] = sum_e dis[col_e]
    (sums include the self-loop edge r->r).
  - Host: shard nodes contiguously (12500/core), bin-pack each core's nodes
    into 98 blocks of 128 balancing per-block message counts; lay out each
    block's messages as CPB chunks of 128 edge slots; ship pre-scaled source
    features Hexp[e] = dis[col]*H[col] (bf16) in chunk-major [slot, chunk*D]
    layout plus per-slot dest keys dkT, per-node sigT/disT, W^T, bias.
  - Device per chunk: S = (iota == dk) one-hot (DVE); zt += Hexp_chunk^T @ S
    (PE, PSUM accum over the block's CPB chunks).  Per block: ztb = bf16(zt)
    (ACT); acc = sigma*bias (DVE preload) + ztb^T... acc += ztb.T @ W^T via
    PE (start=False); out_block = relu(acc * dis) (ACT) -> DMA.
  - No collectives, no dma_gather: GPSIMD stays idle; all DMA is bulk
    contiguous HWDGE.
"""
import numpy as np

N = 100000
D = 128
NCORES = 8
P = 128
NPC_REAL = N // NCORES          # 12500 real nodes per core
NBLK = 98                       # blocks of 128 slots (12544 slots, 44 dummy)
NPC = NBLK * P                  # 12544 slots per core


# ----------------------------------------------------------------- host prep

def _pack_blocks(deg_n, cpb):
    """Bin-pack all N nodes into 784 bins (<=128 each) balancing deg sums.

    deg_n: [N] message counts (in-degree + 1).  Returns [N] bin ids with
    every bin's deg sum <= cpb*128, or None if not achievable.
    """
    cap = cpb * P
    nbins = NCORES * NBLK
    order = np.argsort(-deg_n, kind="stable")
    bins = -np.ones(len(deg_n), dtype=np.int64)
    # snake placement of descending degrees
    for r in range(0, len(order), nbins):
        chunk = order[r:r + nbins]
        ids = np.arange(len(chunk))
        if (r // nbins) % 2 == 1:
            ids = nbins - 1 - ids
        bins[chunk] = ids[:len(chunk)]
    sums = np.bincount(bins, weights=deg_n, minlength=nbins).astype(np.int64)
    cnts = np.bincount(bins, minlength=nbins)
    # greedy fixup: swap items between fullest and emptiest bins
    for _ in range(20000):
        a = int(np.argmax(sums))
        if sums[a] <= cap:
            break
        b = int(np.argmin(sums))
        need = int(sums[a] - cap)
        room = int(cap - sums[b])
        ia = np.where(bins == a)[0]
        ib = np.where(bins == b)[0]
        da, db = deg_n[ia], deg_n[ib]
        # swap (i from a, j from b): diff = da_i - db_j must be >=1 and
        # <= room; prefer the smallest diff >= need, else the largest <= room
        diff = da[:, None] - db[None, :]
        ok = (diff >= 1) & (diff <= room)
        if not ok.any():
            return None
        dd = np.where(ok, diff, -1)
        best = np.where((dd >= min(need, room)) & ok, dd,
                        np.iinfo(np.int64).max)
        if best.min() != np.iinfo(np.int64).max:
            i, j = np.unravel_index(np.argmin(best), best.shape)
        else:
            i, j = np.unravel_index(np.argmax(dd), dd.shape)
        d = int(diff[i, j])
        bins[ia[i]], bins[ib[j]] = b, a
        sums[a] -= d
        sums[b] += d
    if sums.max() > cap or cnts.max() > P:
        return None
    return bins


def _host_prep(H, edge_index, W, b, CPB):
    """Build per-core device inputs; None if CPB chunks/block don't fit."""
    import ml_dtypes
    f32 = np.float32
    bf16 = ml_dtypes.bfloat16
    NC = NBLK * CPB                  # chunks per core

    row = np.asarray(edge_index[0], dtype=np.int64)
    col = np.asarray(edge_index[1], dtype=np.int64)
    H = np.asarray(H, dtype=f32)
    W = np.asarray(W, dtype=f32)
    b = np.asarray(b, dtype=f32)

    deg = (1.0 + np.bincount(row, minlength=N)).astype(f32)
    dis = (1.0 / np.sqrt(deg)).astype(f32)
    # sigma[r] = sum over messages into r (incl self) of dis[col]
    sig = np.bincount(row, weights=dis[col].astype(np.float64),
                      minlength=N).astype(f32) + dis

    disH = (dis[:, None] * H).astype(bf16)        # pre-scaled source features

    deg_i = deg.astype(np.int64)

    iota = np.tile(np.arange(P, dtype=bf16)[None, :], (P, 1))
    WTb = np.ascontiguousarray(W.T).astype(bf16)  # [d, f]
    biasB = np.tile(b[None, :], (P, 1)).astype(f32)

    bins = _pack_blocks(deg_i, CPB)               # global: 784 bins
    if bins is None:
        return None, None
    # node -> global slot: bin b is (core b//NBLK, block b%NBLK)
    order = np.argsort(bins, kind="stable")
    bin_of = bins[order]
    slot_of = np.arange(N) - np.searchsorted(bin_of, bin_of)
    node_pos = np.empty(N, dtype=np.int64)        # core*NPC + blk*P + slot
    node_pos[order] = bin_of * P + slot_of

    allr = np.concatenate([row, np.arange(N, dtype=np.int64)])  # + self loops
    allc = np.concatenate([col, np.arange(N, dtype=np.int64)])
    dst = node_pos[allr]
    ecore = dst // NPC

    in_maps = []
    for c in range(NCORES):
        em = ecore == c
        ec = allc[em]
        dstc = dst[em] - c * NPC
        dblk = dstc // P
        dk = dstc % P
        eorder = np.argsort(dblk, kind="stable")
        dblk_s = dblk[eorder]
        rank = np.arange(len(eorder)) - np.searchsorted(dblk_s, dblk_s)
        assert rank.max() < CPB * P
        cidx = dblk_s * CPB + rank // P              # chunk index
        slot = rank % P

        Hexp3 = np.zeros((NC, P, D), dtype=bf16)
        Hexp3[cidx, slot] = disH[ec[eorder]]
        HexpT = np.ascontiguousarray(
            Hexp3.transpose(1, 0, 2).reshape(P, NC * D))

        dkT = np.full((P, NC), -1.0, dtype=f32)
        dkT[slot, cidx] = dk[eorder].astype(f32)

        nm = (node_pos >= c * NPC) & (node_pos < (c + 1) * NPC)
        npos = node_pos[nm] - c * NPC
        sigT = np.zeros((P, NBLK), dtype=f32)
        disT = np.zeros((P, NBLK), dtype=f32)
        sigT[npos % P, npos // P] = sig[nm]
        disT[npos % P, npos // P] = dis[nm]

        in_maps.append(dict(
            Hexp=HexpT, dkT=np.ascontiguousarray(dkT),
            sigT=np.ascontiguousarray(sigT),
            disT=np.ascontiguousarray(disT),
            WTb=WTb, biasB=biasB, iota=iota,
        ))
    return in_maps, node_pos


# ------------------------------------------------------------- numpy device sim

def _sim_spmd(in_maps, CPB):
    """Numpy mirror of the device program (index-plumbing validation)."""
    import ml_dtypes
    f32 = np.float32
    bf16 = ml_dtypes.bfloat16
    outs = []
    for m in in_maps:
        Hexp = m["Hexp"].astype(f32)          # [128, NC*D]
        dkT = m["dkT"]
        iota = m["iota"].astype(f32)
        out_c = np.zeros((P, NBLK, D), dtype=f32)
        for t in range(NBLK):
            zt = np.zeros((D, P), dtype=f32)  # [d, dst]
            for j in range(CPB):
                c = t * CPB + j
                S = (iota == dkT[:, c:c + 1]).astype(f32)   # [e, dst]
                He = Hexp[:, c * D:(c + 1) * D]             # [e, d]
                zt += He.T @ S
            ztb = zt.astype(bf16).astype(f32)
            acc = m["biasB"] * m["sigT"][:, t:t + 1]        # [dst, f]
            acc = acc + ztb.T @ m["WTb"].astype(f32)
            out_c[:, t, :] = np.maximum(acc * m["disT"][:, t:t + 1], 0.0)
        outs.append(out_c.transpose(1, 0, 2).reshape(NPC, D))
    return outs


# ------------------------------------------------------------- device kernel

_NC_CACHE = {}
_LAST = {}          # exposes (nc, in_maps, CPB) of last kernel() call
GRP = 7             # dest blocks per DMA group (98 = 14*7)
SDVE = 7            # S-builds per block on DVE (rest on ACT)


def _build_nc(CPB):
    import concourse.bacc as bacc
    import concourse.mybir as mybir
    import concourse.tile as tile

    bf = mybir.dt.bfloat16
    f32 = mybir.dt.float32
    NC = NBLK * CPB

    nc = bacc.Bacc("TRN2", target_bir_lowering=False, debug=False,
                   num_devices=NCORES)

    Hexp = nc.dram_tensor("Hexp", [P, NC * D], bf, kind="ExternalInput").ap()
    dkT = nc.dram_tensor("dkT", [P, NC], f32, kind="ExternalInput").ap()
    sigT = nc.dram_tensor("sigT", [P, NBLK], f32, kind="ExternalInput").ap()
    disT = nc.dram_tensor("disT", [P, NBLK], f32, kind="ExternalInput").ap()
    WTb = nc.dram_tensor("WTb", [P, D], bf, kind="ExternalInput").ap()
    biasB = nc.dram_tensor("biasB", [P, D], f32, kind="ExternalInput").ap()
    iota = nc.dram_tensor("iota", [P, P], bf, kind="ExternalInput").ap()
    out = nc.dram_tensor("out", [NPC, D], f32, kind="ExternalOutput").ap()

    with tile.TileContext(nc) as tc:
        with (
            tc.tile_pool(name="const", bufs=1) as const,
            tc.tile_pool(name="hexp", bufs=3) as hpool,
            tc.tile_pool(name="spool", bufs=6) as spool,
            tc.tile_pool(name="ztb", bufs=4) as ztbpool,
            tc.tile_pool(name="ostg", bufs=3) as opool,
            tc.tile_pool(name="zt", bufs=4, space="PSUM") as ztpool,
            tc.tile_pool(name="acc", bufs=4, space="PSUM") as accpool,
        ):
            WTb_s = const.tile([P, D], bf)
            nc.sync.dma_start(out=WTb_s[:], in_=WTb[:])
            biasB_s = const.tile([P, D], f32)
            nc.sync.dma_start(out=biasB_s[:], in_=biasB[:])
            iota7_s = const.tile([P, CPB * P], bf)
            for j in range(CPB):
                nc.sync.dma_start(out=iota7_s[:, j * P:(j + 1) * P],
                                  in_=iota[:])
            sigT_s = const.tile([P, NBLK], f32)
            nc.sync.dma_start(out=sigT_s[:], in_=sigT[:])
            disT_s = const.tile([P, NBLK], f32)
            nc.sync.dma_start(out=disT_s[:], in_=disT[:])
            dkT_s = const.tile([P, NC], f32)
            nc.scalar.dma_start(out=dkT_s[:], in_=dkT[:])

            for g in range(NBLK // GRP):
                eng = nc.sync if g % 2 == 0 else nc.scalar
                hx = hpool.tile([P, GRP * CPB * D], bf, tag="hx",
                                name=f"hx_{g}")
                eng.dma_start(
                    out=hx[:],
                    in_=Hexp[:, g * GRP * CPB * D:(g + 1) * GRP * CPB * D])
                ostg = opool.tile([P, GRP * D], f32, tag="o", name=f"o_{g}")
                for lt in range(GRP):
                    t = g * GRP + lt
                    zt = ztpool.tile([P, P], f32, space="PSUM", tag="zt",
                                     name=f"zt_{t}")
                    # S for all CPB chunks of this block in one DVE op:
                    # S7[p, c, m] = (iota[m] == dk[p, c]) via stride-0 bcast
                    S7 = spool.tile([P, CPB * P], bf, tag="s")
                    dkb = dkT_s[:, t * CPB:(t + 1) * CPB].rearrange(
                        "p (c u) -> p c u", u=1).broadcast_to([P, CPB, P])
                    nc.vector.tensor_tensor(
                        out=S7[:].rearrange("p (c m) -> p c m", c=CPB),
                        in0=iota7_s[:].rearrange("p (c m) -> p c m", c=CPB),
                        in1=dkb, op=mybir.AluOpType.is_equal)
                    for j in range(CPB):
                        nc.tensor.matmul(
                            out=zt[:],
                            lhsT=hx[:, (lt * CPB + j) * D:(lt * CPB + j + 1) * D],
                            rhs=S7[:, j * P:(j + 1) * P],
                            start=(j == 0), stop=(j == CPB - 1))
                    ztb = ztbpool.tile([P, P], bf, tag="ztb")
                    nc.scalar.copy(out=ztb[:], in_=zt[:])
                    acc = accpool.tile([P, D], f32, space="PSUM", tag="acc")
                    nc.vector.tensor_scalar(
                        out=acc[:], in0=biasB_s[:],
                        scalar1=sigT_s[:, t:t + 1], scalar2=None,
                        op0=mybir.AluOpType.mult)
                    nc.tensor.matmul(out=acc[:], lhsT=ztb[:], rhs=WTb_s[:],
                                     start=False, stop=True)
                    nc.scalar.activation(
                        out=ostg[:, lt * D:(lt + 1) * D], in_=acc[:],
                        func=mybir.ActivationFunctionType.Relu,
                        scale=disT_s[:, t:t + 1])
                eng2 = nc.scalar if g % 2 == 0 else nc.sync
                # out is partition-major (flat row = p*NBLK + t)
                eng2.dma_start(
                    out=out[:].rearrange(
                        "(p t) f -> p (t f)", p=P)[:, g * GRP * D:
                                                   (g + 1) * GRP * D],
                    in_=ostg[:])

    nc.finalize()
    return nc


def kernel(H, edge_index, W, b):
    from concourse.bass_utils import run_bass_kernel_spmd

    CPB = 7
    in_maps, node_pos = _host_prep(H, edge_index, W, b, CPB)
    if in_maps is None:
        CPB = 8
        in_maps, node_pos = _host_prep(H, edge_index, W, b, CPB)
        assert in_maps is not None

    if CPB not in _NC_CACHE:
        _NC_CACHE[CPB] = _build_nc(CPB)
    nc = _NC_CACHE[CPB]
    _LAST.update(nc=nc, in_maps=in_maps, CPB=CPB)

    res = run_bass_kernel_spmd(nc, in_maps, list(range(NCORES)))
    # device out is partition-major: flat row = p*NBLK + t -> slot (t, p)
    full = np.empty((NCORES * NPC, D), dtype=np.float32)
    for c in range(NCORES):
        o = res.results[c]["out"].reshape(P, NBLK, D)
        full[c * NPC:(c + 1) * NPC] = o.transpose(1, 0, 2).reshape(NPC, D)
    return np.ascontiguousarray(full[node_pos])


# revision 8
# speedup vs baseline: 7.4901x; 1.0090x over previous
"""GCN layer (message passing) on 8 trn2 NeuronCores.

  out = relu(segment_sum(norm * (H@W.T + b)[col], row)),  norm = d^-1/2[row] d^-1/2[col]
  with self-loops appended; d = 1 + in-degree.

Strategy (SPMD over 8 cores, edges partitioned by destination on host):
  - Aggregate-then-transform (GCN linearity):
      out[r] = relu(dis[r] * (Z[r] @ W.T + sigma[r] * b)),
      Z[r] = sum_{e: dst=r} dis[col_e] * H[col_e],  sigma[r] = sum_e dis[col_e]
    (sums include the self-loop edge r->r).
  - Host: shard nodes contiguously (12500/core), bin-pack each core's nodes
    into 98 blocks of 128 balancing per-block message counts; lay out each
    block's messages as CPB chunks of 128 edge slots; ship pre-scaled source
    features Hexp[e] = dis[col]*H[col] (bf16) in chunk-major [slot, chunk*D]
    layout plus per-slot dest keys dkT, per-node sigT/disT, W^T, bias.
  - Device per chunk: S = (iota == dk) one-hot (DVE); zt += Hexp_chunk^T @ S
    (PE, PSUM accum over the block's CPB chunks).  Per block: ztb = bf16(zt)
    (ACT); acc = sigma*bias (DVE preload) + ztb^T... acc += ztb.T @ W^T via
    PE (start=False); out_block = relu(acc * dis) (ACT) -> DMA.
  - No collectives, no dma_gather: GPSIMD stays idle; all DMA is bulk
    contiguous HWDGE.
"""
import numpy as np

N = 100000
D = 128
NCORES = 8
P = 128
NPC_REAL = N // NCORES          # 12500 real nodes per core
NBLK = 98                       # blocks of 128 slots (12544 slots, 44 dummy)
NPC = NBLK * P                  # 12544 slots per core


# ----------------------------------------------------------------- host prep

def _pack_blocks(deg_n, cpb):
    """Bin-pack all N nodes into 784 bins (<=128 each) balancing deg sums.

    deg_n: [N] message counts (in-degree + 1).  Returns [N] bin ids with
    every bin's deg sum <= cpb*128, or None if not achievable.
    """
    cap = cpb * P
    nbins = NCORES * NBLK
    order = np.argsort(-deg_n, kind="stable")
    bins = -np.ones(len(deg_n), dtype=np.int64)
    # snake placement of descending degrees
    for r in range(0, len(order), nbins):
        chunk = order[r:r + nbins]
        ids = np.arange(len(chunk))
        if (r // nbins) % 2 == 1:
            ids = nbins - 1 - ids
        bins[chunk] = ids[:len(chunk)]
    sums = np.bincount(bins, weights=deg_n, minlength=nbins).astype(np.int64)
    cnts = np.bincount(bins, minlength=nbins)
    # greedy fixup: swap items between fullest and emptiest bins
    for _ in range(20000):
        a = int(np.argmax(sums))
        if sums[a] <= cap:
            break
        b = int(np.argmin(sums))
        need = int(sums[a] - cap)
        room = int(cap - sums[b])
        ia = np.where(bins == a)[0]
        ib = np.where(bins == b)[0]
        da, db = deg_n[ia], deg_n[ib]
        # swap (i from a, j from b): diff = da_i - db_j must be >=1 and
        # <= room; prefer the smallest diff >= need, else the largest <= room
        diff = da[:, None] - db[None, :]
        ok = (diff >= 1) & (diff <= room)
        if not ok.any():
            return None
        dd = np.where(ok, diff, -1)
        best = np.where((dd >= min(need, room)) & ok, dd,
                        np.iinfo(np.int64).max)
        if best.min() != np.iinfo(np.int64).max:
            i, j = np.unravel_index(np.argmin(best), best.shape)
        else:
            i, j = np.unravel_index(np.argmax(dd), dd.shape)
        d = int(diff[i, j])
        bins[ia[i]], bins[ib[j]] = b, a
        sums[a] -= d
        sums[b] += d
    if sums.max() > cap or cnts.max() > P:
        return None
    return bins


def _host_prep(H, edge_index, W, b, CPB):
    """Build per-core device inputs; None if CPB chunks/block don't fit."""
    import ml_dtypes
    f32 = np.float32
    bf16 = ml_dtypes.bfloat16
    NC = NBLK * CPB                  # chunks per core

    row = np.asarray(edge_index[0], dtype=np.int64)
    col = np.asarray(edge_index[1], dtype=np.int64)
    H = np.asarray(H, dtype=f32)
    W = np.asarray(W, dtype=f32)
    b = np.asarray(b, dtype=f32)

    deg = (1.0 + np.bincount(row, minlength=N)).astype(f32)
    dis = (1.0 / np.sqrt(deg)).astype(f32)
    # sigma[r] = sum over messages into r (incl self) of dis[col]
    sig = np.bincount(row, weights=dis[col].astype(np.float64),
                      minlength=N).astype(f32) + dis

    disH = (dis[:, None] * H).astype(bf16)        # pre-scaled source features

    deg_i = deg.astype(np.int64)

    iota = np.tile(np.arange(P, dtype=bf16)[None, :], (P, 1))
    WTb = np.ascontiguousarray(W.T).astype(bf16)  # [d, f]
    biasB = np.tile(b[None, :], (P, 1)).astype(f32)

    bins = _pack_blocks(deg_i, CPB)               # global: 784 bins
    if bins is None:
        return None, None
    # node -> global slot: bin b is (core b//NBLK, block b%NBLK)
    order = np.argsort(bins, kind="stable")
    bin_of = bins[order]
    slot_of = np.arange(N) - np.searchsorted(bin_of, bin_of)
    node_pos = np.empty(N, dtype=np.int64)        # core*NPC + blk*P + slot
    node_pos[order] = bin_of * P + slot_of

    allr = np.concatenate([row, np.arange(N, dtype=np.int64)])  # + self loops
    allc = np.concatenate([col, np.arange(N, dtype=np.int64)])
    dst = node_pos[allr]
    ecore = dst // NPC

    in_maps = []
    for c in range(NCORES):
        em = ecore == c
        ec = allc[em]
        dstc = dst[em] - c * NPC
        dblk = dstc // P
        dk = dstc % P
        eorder = np.argsort(dblk, kind="stable")
        dblk_s = dblk[eorder]
        rank = np.arange(len(eorder)) - np.searchsorted(dblk_s, dblk_s)
        assert rank.max() < CPB * P
        cidx = dblk_s * CPB + rank // P              # chunk index
        slot = rank % P

        Hexp3 = np.zeros((NC, P, D), dtype=bf16)
        Hexp3[cidx, slot] = disH[ec[eorder]]
        HexpT = np.ascontiguousarray(
            Hexp3.transpose(1, 0, 2).reshape(P, NC * D))

        dkT = np.full((P, NC), -1.0, dtype=bf16)
        dkT[slot, cidx] = dk[eorder].astype(bf16)

        nm = (node_pos >= c * NPC) & (node_pos < (c + 1) * NPC)
        npos = node_pos[nm] - c * NPC
        sigT = np.zeros((P, NBLK), dtype=f32)
        disT = np.zeros((P, NBLK), dtype=f32)
        sigT[npos % P, npos // P] = sig[nm]
        disT[npos % P, npos // P] = dis[nm]

        in_maps.append(dict(
            Hexp=HexpT, dkT=np.ascontiguousarray(dkT),
            sigT=np.ascontiguousarray(sigT),
            disT=np.ascontiguousarray(disT),
            WTb=WTb, biasB=biasB, iota=iota,
        ))
    return in_maps, node_pos


# ------------------------------------------------------------- numpy device sim

def _sim_spmd(in_maps, CPB):
    """Numpy mirror of the device program (index-plumbing validation)."""
    import ml_dtypes
    f32 = np.float32
    bf16 = ml_dtypes.bfloat16
    outs = []
    for m in in_maps:
        Hexp = m["Hexp"].astype(f32)          # [128, NC*D]
        dkT = m["dkT"]
        iota = m["iota"].astype(f32)
        out_c = np.zeros((P, NBLK, D), dtype=f32)
        for t in range(NBLK):
            zt = np.zeros((D, P), dtype=f32)  # [d, dst]
            for j in range(CPB):
                c = t * CPB + j
                S = (iota == dkT[:, c:c + 1]).astype(f32)   # [e, dst]
                He = Hexp[:, c * D:(c + 1) * D]             # [e, d]
                zt += He.T @ S
            ztb = zt.astype(bf16).astype(f32)
            acc = m["biasB"] * m["sigT"][:, t:t + 1]        # [dst, f]
            acc = acc + ztb.T @ m["WTb"].astype(f32)
            out_c[:, t, :] = np.maximum(
                acc * m["disT"][:, t:t + 1], 0.0).astype(bf16).astype(f32)
        outs.append(out_c.transpose(1, 0, 2).reshape(NPC, D))
    return outs


# ------------------------------------------------------------- device kernel

_NC_CACHE = {}
_LAST = {}          # exposes (nc, in_maps, CPB) of last kernel() call
GRP = 7             # dest blocks per DMA group (98 = 14*7)
SDVE = 7            # S-builds per block on DVE (rest on ACT)


def _build_nc(CPB):
    import concourse.bacc as bacc
    import concourse.mybir as mybir
    import concourse.tile as tile

    bf = mybir.dt.bfloat16
    f32 = mybir.dt.float32
    NC = NBLK * CPB

    nc = bacc.Bacc("TRN2", target_bir_lowering=False, debug=False,
                   num_devices=NCORES)

    Hexp = nc.dram_tensor("Hexp", [P, NC * D], bf, kind="ExternalInput").ap()
    dkT = nc.dram_tensor("dkT", [P, NC], bf, kind="ExternalInput").ap()
    sigT = nc.dram_tensor("sigT", [P, NBLK], f32, kind="ExternalInput").ap()
    disT = nc.dram_tensor("disT", [P, NBLK], f32, kind="ExternalInput").ap()
    WTb = nc.dram_tensor("WTb", [P, D], bf, kind="ExternalInput").ap()
    biasB = nc.dram_tensor("biasB", [P, D], f32, kind="ExternalInput").ap()
    iota = nc.dram_tensor("iota", [P, P], bf, kind="ExternalInput").ap()
    out = nc.dram_tensor("out", [NPC, D], bf, kind="ExternalOutput").ap()

    with tile.TileContext(nc) as tc:
        with (
            tc.tile_pool(name="const", bufs=1) as const,
            tc.tile_pool(name="hexp", bufs=3) as hpool,
            tc.tile_pool(name="spool", bufs=6) as spool,
            tc.tile_pool(name="ztb", bufs=4) as ztbpool,
            tc.tile_pool(name="ostg", bufs=3) as opool,
            tc.tile_pool(name="zt", bufs=4, space="PSUM") as ztpool,
            tc.tile_pool(name="acc", bufs=4, space="PSUM") as accpool,
        ):
            WTb_s = const.tile([P, D], bf)
            nc.sync.dma_start(out=WTb_s[:], in_=WTb[:])
            biasB_s = const.tile([P, D], f32)
            nc.sync.dma_start(out=biasB_s[:], in_=biasB[:])
            iota7_s = const.tile([P, CPB * P], bf)
            for j in range(CPB):
                nc.sync.dma_start(out=iota7_s[:, j * P:(j + 1) * P],
                                  in_=iota[:])
            sigT_s = const.tile([P, NBLK], f32)
            nc.sync.dma_start(out=sigT_s[:], in_=sigT[:])
            disT_s = const.tile([P, NBLK], f32)
            nc.sync.dma_start(out=disT_s[:], in_=disT[:])
            dkT_s = const.tile([P, NC], bf)
            nc.scalar.dma_start(out=dkT_s[:], in_=dkT[:])

            for g in range(NBLK // GRP):
                eng = nc.sync if g % 2 == 0 else nc.scalar
                hx = hpool.tile([P, GRP * CPB * D], bf, tag="hx",
                                name=f"hx_{g}")
                eng.dma_start(
                    out=hx[:],
                    in_=Hexp[:, g * GRP * CPB * D:(g + 1) * GRP * CPB * D])
                ostg = opool.tile([P, GRP * D], bf, tag="o", name=f"o_{g}")
                for lt in range(GRP):
                    t = g * GRP + lt
                    zt = ztpool.tile([P, P], f32, space="PSUM", tag="zt",
                                     name=f"zt_{t}")
                    # S for all CPB chunks of this block in one DVE op:
                    # S7[p, c, m] = (iota[m] == dk[p, c]) via stride-0 bcast
                    S7 = spool.tile([P, CPB * P], bf, tag="s")
                    dkb = dkT_s[:, t * CPB:(t + 1) * CPB].rearrange(
                        "p (c u) -> p c u", u=1).broadcast_to([P, CPB, P])
                    nc.vector.tensor_tensor(
                        out=S7[:].rearrange("p (c m) -> p c m", c=CPB),
                        in0=iota7_s[:].rearrange("p (c m) -> p c m", c=CPB),
                        in1=dkb, op=mybir.AluOpType.is_equal)
                    for j in range(CPB):
                        nc.tensor.matmul(
                            out=zt[:],
                            lhsT=hx[:, (lt * CPB + j) * D:(lt * CPB + j + 1) * D],
                            rhs=S7[:, j * P:(j + 1) * P],
                            start=(j == 0), stop=(j == CPB - 1))
                    ztb = ztbpool.tile([P, P], bf, tag="ztb")
                    nc.scalar.copy(out=ztb[:], in_=zt[:])
                    acc = accpool.tile([P, D], f32, space="PSUM", tag="acc")
                    nc.vector.tensor_scalar(
                        out=acc[:], in0=biasB_s[:],
                        scalar1=sigT_s[:, t:t + 1], scalar2=None,
                        op0=mybir.AluOpType.mult)
                    nc.tensor.matmul(out=acc[:], lhsT=ztb[:], rhs=WTb_s[:],
                                     start=False, stop=True)
                    nc.scalar.activation(
                        out=ostg[:, lt * D:(lt + 1) * D], in_=acc[:],
                        func=mybir.ActivationFunctionType.Relu,
                        scale=disT_s[:, t:t + 1])
                eng2 = nc.scalar if g % 2 == 0 else nc.sync
                # out is partition-major (flat row = p*NBLK + t)
                eng2.dma_start(
                    out=out[:].rearrange(
                        "(p t) f -> p (t f)", p=P)[:, g * GRP * D:
                                                   (g + 1) * GRP * D],
                    in_=ostg[:])

    nc.finalize()
    return nc


def kernel(H, edge_index, W, b):
    from concourse.bass_utils import run_bass_kernel_spmd

    CPB = 7
    in_maps, node_pos = _host_prep(H, edge_index, W, b, CPB)
    if in_maps is None:
        CPB = 8
        in_maps, node_pos = _host_prep(H, edge_index, W, b, CPB)
        assert in_maps is not None

    if CPB not in _NC_CACHE:
        _NC_CACHE[CPB] = _build_nc(CPB)
    nc = _NC_CACHE[CPB]
    _LAST.update(nc=nc, in_maps=in_maps, CPB=CPB)

    res = run_bass_kernel_spmd(nc, in_maps, list(range(NCORES)))
    # device out is partition-major: flat row = p*NBLK + t -> slot (t, p)
    full = np.empty((NCORES * NPC, D), dtype=np.float32)
    for c in range(NCORES):
        o = np.asarray(res.results[c]["out"], dtype=np.float32).reshape(
            P, NBLK, D)
        full[c * NPC:(c + 1) * NPC] = o.transpose(1, 0, 2).reshape(NPC, D)
    return np.ascontiguousarray(full[node_pos])
